# revision 27
# baseline (speedup 1.0000x reference)
"""Expert-parallel MoE MLP (ExpertMLP) Bass kernel for 8 Trainium2 NeuronCores.

Problem: x[32,4096,256] @ w_fc[32,256,1024] -> gelu(erf) -> @ w_proj[32,1024,256].

Sharding: expert-parallel. Each of the 8 cores gets 4 experts (slices of the
leading axis of every tensor); no cross-core communication. Inside a core, per
expert e:

  1. x[e] ([4096,256], capacity-major) is cast to bf16 (DRAM->DRAM SWDGE cast
     on gpsimd) in 512-row slabs, then each slab is XBar DMA-transposed into
     SBUF as xT [d, c] so the d-contraction of MM1 lies on the partition axis.
     The PE never spends a cycle on transposes.
  2. MM1: hT[h_tile, c_chunk] += w_fc_tile.T @ xT_chunk - w_fc's natural
     [d, h] layout is the stationary operand, so it needs no transpose.
  3. GELU (exact erf form) runs on the ACT engine as the PSUM->SBUF eviction,
     writing bf16 hT tiles.
  4. MM2 uses hT slices as the *stationary* operand and w_proj's natural
     [h, d] layout as the moving operand: out[c_sub, d] += hT_slice.T @
     w_proj_tile. The result lands directly in [capacity, d] orientation, so
     no output transpose is needed.

Scheduling (what makes it fast):
  - Priority-ordered prologue: w_fc[e0] (scalar HWDGE queue) and the cast+
    transpose chain for e0's first slab are enqueued before everything else,
    so MM1 starts ~13us in instead of waiting ~48us for all casts to drain.
  - MM2 of chunk t runs after MM1 of chunk t+1 (one-chunk software pipeline),
    so the ACT-engine GELU of chunk t completes long before MM2 needs it and
    the PE never stalls on the activation.
  - Queue separation: weights go on the Act HWDGE queue, x-casts on the
    gpsimd SWDGE queue, transposes + output stores on the sync HWDGE queue.
    Transposes/casts for expert e+2 are issued inside expert e's loop so
    tile-pool aliasing never head-of-line-blocks the store stream.
"""

import numpy as np
from contextlib import ExitStack

import bass_rust as _br
import concourse.bass as bass
import concourse.tile as tile
from concourse import mybir
from concourse.bass_utils import run_bass_kernel_spmd

E, CAP, D, H = 32, 4096, 256, 1024
N_CORES = 8
E_PER = E // N_CORES  # 4 experts per core
P = 128
F32 = mybir.dt.float32
F32R = mybir.dt.float32r
BF16 = mybir.dt.bfloat16

KD = D // P        # 2 k-tiles in MM1's contraction
KH = H // P        # 8 k-tiles in MM2's contraction
NC_CHUNK = 512     # capacity chunk processed per MM1/MM2 round == slab size
N_CHUNKS = CAP // NC_CHUNK
H_TILES = H // P
HPACK = 2          # h_tiles packed per PSUM tile / GELU call
NS = NC_CHUNK // P
T_CHUNKS = E_PER * N_CHUNKS  # 32 global (expert, chunk) rounds


def _fix_waits(nc):
    """walrus here accepts only one sync wait per instruction; hoist excess
    waits onto standalone EventSemaphore instructions inserted before the
    offender (same engine => same sequencer order)."""
    for fn in nc.m.functions:
        for bb in fn.blocks:
            new = []
            changed = False
            for inst in bb.instructions:
                si = inst.sync_info
                if si is not None and len(si.on_wait) > 1:
                    waits = list(si.on_wait)
                    for w in waits[:-1]:
                        ev = mybir.InstEventSemaphore(
                            name=nc.get_next_instruction_name()
                        )
                        ev.engine = inst.engine
                        ev.sync_info = _br.SyncInfo(on_wait=[w], on_update=[])
                        nc.register_instruction(ev)
                        new.append(ev)
                    inst.sync_info = _br.SyncInfo(
                        on_wait=waits[-1:], on_update=list(si.on_update)
                    )
                    changed = True
                new.append(inst)
            if changed:
                bb.instructions = new


def _build():
    nc = bass.Bass(trn_type="TRN2", target_bir_lowering=False, debug=False)
    x = nc.dram_tensor("x", [E_PER, CAP, D], F32, kind="ExternalInput").ap()
    w_fc = nc.dram_tensor("w_fc", [E_PER, D, H], F32, kind="ExternalInput").ap()
    w_proj = nc.dram_tensor("w_proj", [E_PER, H, D], F32, kind="ExternalInput").ap()
    out = nc.dram_tensor("out", [E_PER, CAP, D], F32, kind="ExternalOutput").ap()

    with tile.TileContext(nc) as tc, ExitStack() as ctx:
        # 2 experts' worth of xT slabs in flight; ring aliasing gates the
        # transposes of expert e+2 on MM1 of expert e having consumed the slab.
        # x staging pools, one 512-row slab at a time:
        #   xsf: f32 slab straight off HBM     [128, NS, 256] (4 KB/part)
        #   xsb: bf16 copy (DVE cast)          [128, NS, 256] (2 KB/part)
        #   xtp: XBar-transposed xT blocks     [128, NS*KD, 128] (2 KB/part)
        # Ring aliasing paces the loads: load i+6 waits for cast i, cast i+6
        # waits for transpose i, transpose i+16 waits for MM1 reads of the
        # expert two ahead.
        xsf = ctx.enter_context(tc.tile_pool(name="xsf", bufs=6))
        xsb = ctx.enter_context(tc.tile_pool(name="xsb", bufs=6))
        xtp = ctx.enter_context(tc.tile_pool(name="xtp", bufs=16))
        wload = ctx.enter_context(tc.tile_pool(name="wload", bufs=2))
        wfc_p = ctx.enter_context(tc.tile_pool(name="wfc", bufs=2))
        wproj_p = ctx.enter_context(tc.tile_pool(name="wproj", bufs=2))
        ht_p = ctx.enter_context(tc.tile_pool(name="ht", bufs=8))
        out_p = ctx.enter_context(tc.tile_pool(name="outp", bufs=3))
        ps_h = ctx.enter_context(tc.tile_pool(name="ps_h", bufs=2, space="PSUM"))
        ps_o = ctx.enter_context(tc.tile_pool(name="ps_o", bufs=4, space="PSUM"))

        def load_weights(e):
            # raw f32 loads on the Act HWDGE queue (kept clear of the cast
            # and transpose streams), cast to bf16 on the idle DVE.
            wfc_raw = wload.tile([P, KD, H], F32, tag="wl")
            nc.scalar.dma_start(wfc_raw[:], w_fc[e].rearrange("(k p) h -> p k h", p=P))
            wfc = wfc_p.tile([P, KD, H], BF16, tag="wfc")
            nc.vector.tensor_copy(wfc[:], wfc_raw[:])
            wproj_raw = wload.tile([P, KH, D], F32, tag="wl")
            nc.scalar.dma_start(
                wproj_raw[:], w_proj[e].rearrange("(k p) d -> p k d", p=P)
            )
            wproj = wproj_p.tile([P, KH, D], BF16, tag="wproj")
            nc.vector.tensor_copy(wproj[:], wproj_raw[:])
            return wfc, wproj

        # per-(expert, chunk): MM1 moving-operand views [128, NS, 128], one
        # per k-tile
        xslices = [[None] * N_CHUNKS for _ in range(E_PER)]

        def stage_slab(e, s, queue):
            """stage one 512-row slab of x[e], entirely through SBUF:
            1. DMA the f32 slab to SBUF partition-blocked: [128, b, 256].
            2. Cast f32 -> bf16 on the (idle) DVE.
            3. ONE wide SBUF->SBUF XBar transpose [128, 1024] ->
               [128, (b k), 128]: each 128-column group of the source becomes
               one output block, so all NS*KD transposed 128x128 blocks land
               in one instruction. No HBM traffic at all for the transpose.
            MM1 then reads k-tile views [128, b, 128] (strided middle dim)."""
            rs = slice(s * NC_CHUNK, (s + 1) * NC_CHUNK)
            xf = xsf.tile([P, NS, D], F32, tag="xf", name=f"xf{e}_{s}")
            queue.dma_start(xf[:], x[e][rs].rearrange("(b p) d -> p b d", p=P))
            xb = xsb.tile([P, NS, D], BF16, tag="xb", name=f"xb{e}_{s}")
            nc.vector.tensor_copy(xb[:], xf[:])
            xt = xtp.tile([P, NS * KD, P], BF16, tag="xt", name=f"xt{e}_{s}")
            nc.sync.dma_start_transpose(xt[:], xb[:])
            xkv = xt[:].rearrange("p (b k) c -> p k b c", k=KD)
            xslices[e][s] = [xkv[:, k] for k in range(KD)]

        # ---- prologue: e0's weights first, then all slab chains in
        # consumption order. The first slabs load on the sync queue (idle at
        # startup - the Act queue is busy with weights), the rest on Act.
        # Tile-ring aliasing paces everything against consumption.
        w = [None] * E_PER
        w[0] = load_weights(0)
        for e in range(E_PER):
            for s in range(N_CHUNKS):
                queue = nc.sync if e == 0 else nc.scalar
                stage_slab(e, s, queue)

        # pending MM2 work: (e, nci, ht_tiles) of the previous chunk round
        pend = None

        def run_mm2(p_e, p_nci, p_ht, last):
            wproj_t = w[p_e][1]
            psos = [
                ps_o.tile([P, 2 * D], F32, tag="pso",
                          name=f"pso{p_e}_{p_nci}_{i}")
                for i in range(NS)
            ]
            ob = out_p.tile([P, NS, D], F32, tag="ob")
            order = (
                [(s, k) for s in range(NS) for k in range(KH)]
                if last else
                [(s, k) for k in range(KH) for s in range(NS)]
            )
            for s, k in order:
                nc.tensor.matmul(
                    psos[s][:, :D],
                    p_ht[k // HPACK][:, k % HPACK, s * P:(s + 1) * P],
                    wproj_t[:, k, :],
                    start=(k == 0),
                    stop=(k == KH - 1),
                )
                if last and k == KH - 1:
                    # final round: per-subtile eviction+store so the output
                    # tail overlaps the last matmuls
                    nc.vector.tensor_copy(ob[:, s, :], psos[s][:, :D])
                    nc.scalar.dma_start(
                        out[p_e, p_nci * NC_CHUNK + s * P:
                            p_nci * NC_CHUNK + (s + 1) * P, :],
                        ob[:, s, :],
                    )
            if not last:
                for s, pso in enumerate(psos):
                    nc.vector.tensor_copy(ob[:, s, :], pso[:, :D])
                csl = slice(p_nci * NC_CHUNK, (p_nci + 1) * NC_CHUNK)
                nc.scalar.dma_start(
                    out[p_e, csl, :].rearrange("(s p) d -> p s d", p=P), ob[:]
                )

        for t in range(T_CHUNKS + 1):
            if t < T_CHUNKS:
                e, nci = divmod(t, N_CHUNKS)
                if nci == 0 and e + 1 < E_PER:
                    w[e + 1] = load_weights(e + 1)
                wfc_t = w[e][0]
                # ---- MM1 -> GELU for chunk t ----
                # MM1 accumulates HPACK h_tiles into one 2-bank PSUM tile so
                # GELU evicts in wide ACTIVATE calls; hT is written bf16 so
                # MM2's per-matmul weight loads run at 2-byte FWL speed.
                xk = xslices[e][nci]
                ht_tiles = []
                for hp in range(H_TILES // HPACK):
                    psh = ps_h.tile([P, HPACK, NC_CHUNK], F32, tag="psh")
                    for j in range(HPACK):
                        hi = hp * HPACK + j
                        for k in range(KD):
                            nc.tensor.matmul(
                                psh[:, j, :],
                                wfc_t[:, k, hi * P:(hi + 1) * P],
                                xk[k],
                                start=(k == 0),
                                stop=(k == KD - 1),
                            )
                    ht = ht_p.tile([P, HPACK, NC_CHUNK], BF16, tag="ht")
                    nc.scalar.activation(
                        ht[:], psh[:], mybir.ActivationFunctionType.Gelu
                    )
                    ht_tiles.append(ht)

            # ---- MM2 for the previous chunk round (one-chunk delay: its
            # GELUs completed during this round's MM1, so the PE never
            # waits on the ACT engine) ----
            if pend is not None:
                p_e, p_nci, p_ht = pend
                run_mm2(p_e, p_nci, p_ht, last=(t == T_CHUNKS))
            pend = (e, nci, ht_tiles) if t < T_CHUNKS else None

    _fix_waits(nc)
    return nc


_CACHE = {}


def _get_nc():
    if "nc" not in _CACHE:
        _CACHE["nc"] = _build()
    return _CACHE["nc"]


def kernel(x, w_fc, w_proj, trace=False):
    assert x.shape == (E, CAP, D) and w_fc.shape == (E, D, H)
    assert w_proj.shape == (E, H, D)
    nc = _get_nc()
    x = np.ascontiguousarray(x, dtype=np.float32)
    w_fc = np.ascontiguousarray(w_fc, dtype=np.float32)
    w_proj = np.ascontiguousarray(w_proj, dtype=np.float32)
    in_maps = [
        {
            "x": x[i * E_PER:(i + 1) * E_PER],
            "w_fc": w_fc[i * E_PER:(i + 1) * E_PER],
            "w_proj": w_proj[i * E_PER:(i + 1) * E_PER],
        }
        for i in range(N_CORES)
    ]
    res = run_bass_kernel_spmd(nc, in_maps, list(range(N_CORES)), trace=trace)
    out = np.concatenate([r["out"] for r in res.results], axis=0)
    if trace:
        kernel.last_results = res
    return out


# revision 32
# speedup vs baseline: 1.0534x; 1.0534x over previous
"""Expert-parallel MoE MLP (ExpertMLP) Bass kernel for 8 Trainium2 NeuronCores.

Problem: x[32,4096,256] @ w_fc[32,256,1024] -> gelu(erf) -> @ w_proj[32,1024,256].

Sharding: expert-parallel. Each of the 8 cores gets 4 experts (slices of the
leading axis of every tensor); no cross-core communication. Inside a core, per
expert e:

  1. x[e] ([4096,256], capacity-major) is cast to bf16 (DRAM->DRAM SWDGE cast
     on gpsimd) in 512-row slabs, then each slab is XBar DMA-transposed into
     SBUF as xT [d, c] so the d-contraction of MM1 lies on the partition axis.
     The PE never spends a cycle on transposes.
  2. MM1: hT[h_tile, c_chunk] += w_fc_tile.T @ xT_chunk - w_fc's natural
     [d, h] layout is the stationary operand, so it needs no transpose.
  3. GELU (exact erf form) runs on the ACT engine as the PSUM->SBUF eviction,
     writing bf16 hT tiles.
  4. MM2 uses hT slices as the *stationary* operand and w_proj's natural
     [h, d] layout as the moving operand: out[c_sub, d] += hT_slice.T @
     w_proj_tile. The result lands directly in [capacity, d] orientation, so
     no output transpose is needed.

Scheduling (what makes it fast):
  - Priority-ordered prologue: w_fc[e0] (scalar HWDGE queue) and the cast+
    transpose chain for e0's first slab are enqueued before everything else,
    so MM1 starts ~13us in instead of waiting ~48us for all casts to drain.
  - MM2 of chunk t runs after MM1 of chunk t+1 (one-chunk software pipeline),
    so the ACT-engine GELU of chunk t completes long before MM2 needs it and
    the PE never stalls on the activation.
  - Queue separation: weights go on the Act HWDGE queue, x-casts on the
    gpsimd SWDGE queue, transposes + output stores on the sync HWDGE queue.
    Transposes/casts for expert e+2 are issued inside expert e's loop so
    tile-pool aliasing never head-of-line-blocks the store stream.
"""

import numpy as np
from contextlib import ExitStack

import bass_rust as _br
import concourse.bass as bass
import concourse.tile as tile
from concourse import mybir
from concourse.bass_utils import run_bass_kernel_spmd

E, CAP, D, H = 32, 4096, 256, 1024
N_CORES = 8
E_PER = E // N_CORES  # 4 experts per core
P = 128
F32 = mybir.dt.float32
F32R = mybir.dt.float32r
BF16 = mybir.dt.bfloat16

KD = D // P        # 2 k-tiles in MM1's contraction
KH = H // P        # 8 k-tiles in MM2's contraction
NC_CHUNK = 512     # capacity chunk processed per MM1/MM2 round == slab size
N_CHUNKS = CAP // NC_CHUNK
H_TILES = H // P
HPACK = 2          # h_tiles packed per PSUM tile / GELU call
NS = NC_CHUNK // P
T_CHUNKS = E_PER * N_CHUNKS  # 32 global (expert, chunk) rounds


def _fix_waits(nc):
    """walrus here accepts only one sync wait per instruction; hoist excess
    waits onto standalone EventSemaphore instructions inserted before the
    offender (same engine => same sequencer order)."""
    for fn in nc.m.functions:
        for bb in fn.blocks:
            new = []
            changed = False
            for inst in bb.instructions:
                si = inst.sync_info
                if si is not None and len(si.on_wait) > 1:
                    waits = list(si.on_wait)
                    for w in waits[:-1]:
                        ev = mybir.InstEventSemaphore(
                            name=nc.get_next_instruction_name()
                        )
                        ev.engine = inst.engine
                        ev.sync_info = _br.SyncInfo(on_wait=[w], on_update=[])
                        nc.register_instruction(ev)
                        new.append(ev)
                    inst.sync_info = _br.SyncInfo(
                        on_wait=waits[-1:], on_update=list(si.on_update)
                    )
                    changed = True
                new.append(inst)
            if changed:
                bb.instructions = new


def _build():
    nc = bass.Bass(trn_type="TRN2", target_bir_lowering=False, debug=False)
    x = nc.dram_tensor("x", [E_PER, CAP, D], F32, kind="ExternalInput").ap()
    w_fc = nc.dram_tensor("w_fc", [E_PER, D, H], F32, kind="ExternalInput").ap()
    w_proj = nc.dram_tensor("w_proj", [E_PER, H, D], F32, kind="ExternalInput").ap()
    out = nc.dram_tensor("out", [E_PER, CAP, D], F32, kind="ExternalOutput").ap()

    with tile.TileContext(nc) as tc, ExitStack() as ctx:
        # 2 experts' worth of xT slabs in flight; ring aliasing gates the
        # transposes of expert e+2 on MM1 of expert e having consumed the slab.
        # x staging pools, one 512-row slab at a time:
        #   xsf: f32 slab straight off HBM     [128, NS, 256] (4 KB/part)
        #   xsb: bf16 copy (DVE cast)          [128, NS, 256] (2 KB/part)
        #   xtp: XBar-transposed xT blocks     [128, NS*KD, 128] (2 KB/part)
        # Ring aliasing paces the loads: load i+6 waits for cast i, cast i+6
        # waits for transpose i, transpose i+16 waits for MM1 reads of the
        # expert two ahead.
        xsf = ctx.enter_context(tc.tile_pool(name="xsf", bufs=6))
        xsb = ctx.enter_context(tc.tile_pool(name="xsb", bufs=6))
        xtp = ctx.enter_context(tc.tile_pool(name="xtp", bufs=16))
        wload = ctx.enter_context(tc.tile_pool(name="wload", bufs=4))
        wfc_p = ctx.enter_context(tc.tile_pool(name="wfc", bufs=2))
        wproj_p = ctx.enter_context(tc.tile_pool(name="wproj", bufs=2))
        ht_p = ctx.enter_context(tc.tile_pool(name="ht", bufs=8))
        out_p = ctx.enter_context(tc.tile_pool(name="outp", bufs=3))
        ps_h = ctx.enter_context(tc.tile_pool(name="ps_h", bufs=2, space="PSUM"))
        ps_o = ctx.enter_context(tc.tile_pool(name="ps_o", bufs=4, space="PSUM"))

        def load_weights(e):
            # raw f32 loads on the Act HWDGE queue (kept clear of the cast
            # and transpose streams), cast to bf16 on the idle DVE.
            wfc_raw = wload.tile([P, KD, H], F32, tag="wl")
            nc.scalar.dma_start(wfc_raw[:], w_fc[e].rearrange("(k p) h -> p k h", p=P))
            wfc = wfc_p.tile([P, KD, H], BF16, tag="wfc")
            nc.vector.tensor_copy(wfc[:], wfc_raw[:])
            wproj_raw = wload.tile([P, KH, D], F32, tag="wl")
            nc.scalar.dma_start(
                wproj_raw[:], w_proj[e].rearrange("(k p) d -> p k d", p=P)
            )
            wproj = wproj_p.tile([P, KH, D], BF16, tag="wproj")
            nc.vector.tensor_copy(wproj[:], wproj_raw[:])
            return wfc, wproj

        # per-(expert, chunk): MM1 moving-operand views [128, NS, 128], one
        # per k-tile
        xslices = [[None] * N_CHUNKS for _ in range(E_PER)]

        def stage_slab(e, s, queue):
            """stage one 512-row slab of x[e], entirely through SBUF:
            1. DMA the f32 slab to SBUF partition-blocked: [128, b, 256].
            2. Cast f32 -> bf16 on the (idle) DVE.
            3. ONE wide SBUF->SBUF XBar transpose [128, 1024] ->
               [128, (b k), 128]: each 128-column group of the source becomes
               one output block, so all NS*KD transposed 128x128 blocks land
               in one instruction. No HBM traffic at all for the transpose.
            MM1 then reads k-tile views [128, b, 128] (strided middle dim)."""
            rs = slice(s * NC_CHUNK, (s + 1) * NC_CHUNK)
            xf = xsf.tile([P, NS, D], F32, tag="xf", name=f"xf{e}_{s}")
            queue.dma_start(xf[:], x[e][rs].rearrange("(b p) d -> p b d", p=P))
            xb = xsb.tile([P, NS, D], BF16, tag="xb", name=f"xb{e}_{s}")
            nc.vector.tensor_copy(xb[:], xf[:])
            xt = xtp.tile([P, NS * KD, P], BF16, tag="xt", name=f"xt{e}_{s}")
            nc.sync.dma_start_transpose(xt[:], xb[:])
            xkv = xt[:].rearrange("p (b k) c -> p k b c", k=KD)
            xslices[e][s] = [xkv[:, k] for k in range(KD)]

        # ---- prologue: everything is issued here, weights interleaved per
        # expert so the in-order DVE stream goes cast(w_e), slabs(e),
        # cast(w_e+1), ... and never makes anyone wait long. Expert 0's
        # slabs load on the sync queue (idle at startup - the Act queue is
        # busy with w0), the rest on Act. Tile-ring aliasing paces all
        # staging against consumption.
        w = [None] * E_PER
        for e in range(E_PER):
            w[e] = load_weights(e)
            for s in range(N_CHUNKS):
                queue = nc.sync if e == 0 else nc.scalar
                stage_slab(e, s, queue)

        # pending MM2 work: (e, nci, ht_tiles) of the previous chunk round
        pend = None

        def run_mm2(p_e, p_nci, p_ht, last):
            wproj_t = w[p_e][1]
            psos = [
                ps_o.tile([P, 2 * D], F32, tag="pso",
                          name=f"pso{p_e}_{p_nci}_{i}")
                for i in range(NS)
            ]
            ob = out_p.tile([P, NS, D], F32, tag="ob")
            order = (
                [(s, k) for s in range(NS) for k in range(KH)]
                if last else
                [(s, k) for k in range(KH) for s in range(NS)]
            )
            for s, k in order:
                nc.tensor.matmul(
                    psos[s][:, :D],
                    p_ht[k // HPACK][:, k % HPACK, s * P:(s + 1) * P],
                    wproj_t[:, k, :],
                    start=(k == 0),
                    stop=(k == KH - 1),
                )
                if last and k == KH - 1:
                    # final round: per-subtile eviction+store so the output
                    # tail overlaps the last matmuls. The whole output path
                    # (PSUM eviction + store) lives on the otherwise-idle
                    # gpsimd engine/SWDGE queue, so it never queues behind
                    # the x-staging casts (DVE) or loads (Act/sync queues).
                    nc.scalar.activation(
                        ob[:, s, :], psos[s][:, :D],
                        mybir.ActivationFunctionType.Copy,
                    )
                    nc.gpsimd.dma_start(
                        out[p_e, p_nci * NC_CHUNK + s * P:
                            p_nci * NC_CHUNK + (s + 1) * P, :],
                        ob[:, s, :],
                    )
            if not last:
                for s, pso in enumerate(psos):
                    nc.scalar.activation(
                        ob[:, s, :], pso[:, :D],
                        mybir.ActivationFunctionType.Copy,
                    )
                csl = slice(p_nci * NC_CHUNK, (p_nci + 1) * NC_CHUNK)
                nc.gpsimd.dma_start(
                    out[p_e, csl, :].rearrange("(s p) d -> p s d", p=P), ob[:]
                )

        for t in range(T_CHUNKS + 1):
            if t < T_CHUNKS:
                e, nci = divmod(t, N_CHUNKS)
                wfc_t = w[e][0]
                # ---- MM1 -> GELU for chunk t ----
                # MM1 accumulates HPACK h_tiles into one 2-bank PSUM tile so
                # GELU evicts in wide ACTIVATE calls; hT is written bf16 so
                # MM2's per-matmul weight loads run at 2-byte FWL speed.
                xk = xslices[e][nci]
                ht_tiles = []
                for hp in range(H_TILES // HPACK):
                    psh = ps_h.tile([P, HPACK, NC_CHUNK], F32, tag="psh")
                    for j in range(HPACK):
                        hi = hp * HPACK + j
                        for k in range(KD):
                            nc.tensor.matmul(
                                psh[:, j, :],
                                wfc_t[:, k, hi * P:(hi + 1) * P],
                                xk[k],
                                start=(k == 0),
                                stop=(k == KD - 1),
                            )
                    ht = ht_p.tile([P, HPACK, NC_CHUNK], BF16, tag="ht")
                    nc.scalar.activation(
                        ht[:], psh[:], mybir.ActivationFunctionType.Gelu
                    )
                    ht_tiles.append(ht)

            # ---- MM2 for the previous chunk round (one-chunk delay: its
            # GELUs completed during this round's MM1, so the PE never
            # waits on the ACT engine) ----
            if pend is not None:
                p_e, p_nci, p_ht = pend
                run_mm2(p_e, p_nci, p_ht, last=(t == T_CHUNKS))
            pend = (e, nci, ht_tiles) if t < T_CHUNKS else None

    _fix_waits(nc)
    return nc


_CACHE = {}


def _get_nc():
    if "nc" not in _CACHE:
        _CACHE["nc"] = _build()
    return _CACHE["nc"]


def kernel(x, w_fc, w_proj, trace=False):
    assert x.shape == (E, CAP, D) and w_fc.shape == (E, D, H)
    assert w_proj.shape == (E, H, D)
    nc = _get_nc()
    x = np.ascontiguousarray(x, dtype=np.float32)
    w_fc = np.ascontiguousarray(w_fc, dtype=np.float32)
    w_proj = np.ascontiguousarray(w_proj, dtype=np.float32)
    in_maps = [
        {
            "x": x[i * E_PER:(i + 1) * E_PER],
            "w_fc": w_fc[i * E_PER:(i + 1) * E_PER],
            "w_proj": w_proj[i * E_PER:(i + 1) * E_PER],
        }
        for i in range(N_CORES)
    ]
    res = run_bass_kernel_spmd(nc, in_maps, list(range(N_CORES)), trace=trace)
    out = np.concatenate([r["out"] for r in res.results], axis=0)
    if trace:
        kernel.last_results = res
    return out


# revision 36
# speedup vs baseline: 1.2305x; 1.1681x over previous
"""Expert-parallel MoE MLP (ExpertMLP) Bass kernel for 8 Trainium2 NeuronCores.

Problem: x[32,4096,256] @ w_fc[32,256,1024] -> gelu(erf) -> @ w_proj[32,1024,256].

Sharding: expert-parallel. Each of the 8 cores gets 4 experts (slices of the
leading axis of every tensor); no cross-core communication. Inside a core, per
expert e:

  1. x[e] ([4096,256], capacity-major) is cast to bf16 (DRAM->DRAM SWDGE cast
     on gpsimd) in 512-row slabs, then each slab is XBar DMA-transposed into
     SBUF as xT [d, c] so the d-contraction of MM1 lies on the partition axis.
     The PE never spends a cycle on transposes.
  2. MM1: hT[h_tile, c_chunk] += w_fc_tile.T @ xT_chunk - w_fc's natural
     [d, h] layout is the stationary operand, so it needs no transpose.
  3. GELU (exact erf form) runs on the ACT engine as the PSUM->SBUF eviction,
     writing bf16 hT tiles.
  4. MM2 uses hT slices as the *stationary* operand and w_proj's natural
     [h, d] layout as the moving operand: out[c_sub, d] += hT_slice.T @
     w_proj_tile. The result lands directly in [capacity, d] orientation, so
     no output transpose is needed.

Scheduling (what makes it fast):
  - Priority-ordered prologue: w_fc[e0] (scalar HWDGE queue) and the cast+
    transpose chain for e0's first slab are enqueued before everything else,
    so MM1 starts ~13us in instead of waiting ~48us for all casts to drain.
  - MM2 of chunk t runs after MM1 of chunk t+1 (one-chunk software pipeline),
    so the ACT-engine GELU of chunk t completes long before MM2 needs it and
    the PE never stalls on the activation.
  - Queue separation: weights go on the Act HWDGE queue, x-casts on the
    gpsimd SWDGE queue, transposes + output stores on the sync HWDGE queue.
    Transposes/casts for expert e+2 are issued inside expert e's loop so
    tile-pool aliasing never head-of-line-blocks the store stream.
"""

import numpy as np
from contextlib import ExitStack

import bass_rust as _br
import concourse.bass as bass
import concourse.tile as tile
from concourse import mybir
from concourse.bass_utils import run_bass_kernel_spmd

E, CAP, D, H = 32, 4096, 256, 1024
N_CORES = 8
E_PER = E // N_CORES  # 4 experts per core
P = 128
F32 = mybir.dt.float32
F32R = mybir.dt.float32r
BF16 = mybir.dt.bfloat16

KD = D // P        # 2 k-tiles in MM1's contraction
KH = H // P        # 8 k-tiles in MM2's contraction
NC_CHUNK = 512     # capacity chunk processed per MM1/MM2 round == slab size
N_CHUNKS = CAP // NC_CHUNK
H_TILES = H // P
HPACK = 2          # h_tiles packed per PSUM tile / GELU call
NS = NC_CHUNK // P
T_CHUNKS = E_PER * N_CHUNKS  # 32 global (expert, chunk) rounds


def _fix_waits(nc):
    """walrus here accepts only one sync wait per instruction; hoist excess
    waits onto standalone EventSemaphore instructions inserted before the
    offender (same engine => same sequencer order)."""
    for fn in nc.m.functions:
        for bb in fn.blocks:
            new = []
            changed = False
            for inst in bb.instructions:
                si = inst.sync_info
                if si is not None and len(si.on_wait) > 1:
                    waits = list(si.on_wait)
                    for w in waits[:-1]:
                        ev = mybir.InstEventSemaphore(
                            name=nc.get_next_instruction_name()
                        )
                        ev.engine = inst.engine
                        ev.sync_info = _br.SyncInfo(on_wait=[w], on_update=[])
                        nc.register_instruction(ev)
                        new.append(ev)
                    inst.sync_info = _br.SyncInfo(
                        on_wait=waits[-1:], on_update=list(si.on_update)
                    )
                    changed = True
                new.append(inst)
            if changed:
                bb.instructions = new


def _build():
    nc = bass.Bass(trn_type="TRN2", target_bir_lowering=False, debug=False)
    x = nc.dram_tensor("x", [E_PER, CAP, D], F32, kind="ExternalInput").ap()
    w_fc = nc.dram_tensor("w_fc", [E_PER, D, H], F32, kind="ExternalInput").ap()
    w_proj = nc.dram_tensor("w_proj", [E_PER, H, D], F32, kind="ExternalInput").ap()
    out = nc.dram_tensor("out", [E_PER, CAP, D], F32, kind="ExternalOutput").ap()

    with tile.TileContext(nc) as tc, ExitStack() as ctx:
        # 2 experts' worth of xT slabs in flight; ring aliasing gates the
        # transposes of expert e+2 on MM1 of expert e having consumed the slab.
        # x staging pools, one 512-row slab at a time:
        #   xsf: f32 slab straight off HBM     [128, NS, 256] (4 KB/part)
        #   xsb: bf16 copy (DVE cast)          [128, NS, 256] (2 KB/part)
        #   xtp: XBar-transposed xT blocks     [128, NS*KD, 128] (2 KB/part)
        # Ring aliasing paces the loads: load i+6 waits for cast i, cast i+6
        # waits for transpose i, transpose i+16 waits for MM1 reads of the
        # expert two ahead.
        xsf = ctx.enter_context(tc.tile_pool(name="xsf", bufs=10))
        xsb = ctx.enter_context(tc.tile_pool(name="xsb", bufs=8))
        xtp = ctx.enter_context(tc.tile_pool(name="xtp", bufs=16))
        wload = ctx.enter_context(tc.tile_pool(name="wload", bufs=4))
        wfc_p = ctx.enter_context(tc.tile_pool(name="wfc", bufs=2))
        wproj_p = ctx.enter_context(tc.tile_pool(name="wproj", bufs=2))
        ht_p = ctx.enter_context(tc.tile_pool(name="ht", bufs=8))
        out_p = ctx.enter_context(tc.tile_pool(name="outp", bufs=3))
        ps_h = ctx.enter_context(tc.tile_pool(name="ps_h", bufs=2, space="PSUM"))
        ps_o = ctx.enter_context(tc.tile_pool(name="ps_o", bufs=4, space="PSUM"))

        def load_weights(e):
            # raw f32 loads on the Act HWDGE queue (kept clear of the cast
            # and transpose streams), cast to bf16 on the idle DVE.
            wfc_raw = wload.tile([P, KD, H], F32, tag="wl")
            nc.scalar.dma_start(wfc_raw[:], w_fc[e].rearrange("(k p) h -> p k h", p=P))
            wfc = wfc_p.tile([P, KD, H], BF16, tag="wfc")
            nc.vector.tensor_copy(wfc[:], wfc_raw[:])
            wproj_raw = wload.tile([P, KH, D], F32, tag="wl")
            nc.scalar.dma_start(
                wproj_raw[:], w_proj[e].rearrange("(k p) d -> p k d", p=P)
            )
            wproj = wproj_p.tile([P, KH, D], BF16, tag="wproj")
            nc.vector.tensor_copy(wproj[:], wproj_raw[:])
            return wfc, wproj

        # x staging, one 512-row slab at a time, in three engine-disjoint
        # stages (global slab index g = e*N_CHUNKS + s = chunk index):
        #   load: DMA the f32 slab to SBUF partition-blocked [128, b, 256]
        #   cast: f32 -> bf16 on the DVE
        #   tpose: ONE wide SBUF->SBUF XBar transpose [128, 1024] ->
        #          [128, (b k), 128] - each 128-column group of the source
        #          becomes one transposed output block, so all NS*KD
        #          128x128 blocks land in one instruction, and the transpose
        #          never touches HBM.
        # MM1 then reads k-tile views [128, b, 128] (strided middle dim).
        # Staging is issued IN-LOOP, staggered (load g+8 / cast g+7 /
        # transpose g+6 at chunk g): every in-order engine sees staging
        # interleaved with its consumption-time work, so no long prologue
        # stream ever blocks a later instruction on the same engine.
        xslices = [[None] * N_CHUNKS for _ in range(E_PER)]
        xbt = [None] * T_CHUNKS

        def stage_load(g, queue):
            e, s = divmod(g, N_CHUNKS)
            rs = slice(s * NC_CHUNK, (s + 1) * NC_CHUNK)
            xf = xsf.tile([P, NS, D], F32, tag="xf", name=f"xf{g}")
            queue.dma_start(xf[:], x[e][rs].rearrange("(b p) d -> p b d", p=P))
            xbt[g] = xf

        def stage_cast(g):
            xb = xsb.tile([P, NS, D], BF16, tag="xb", name=f"xb{g}")
            nc.vector.tensor_copy(xb[:], xbt[g][:])
            xbt[g] = xb

        def stage_tpose(g):
            e, s = divmod(g, N_CHUNKS)
            xt = xtp.tile([P, NS * KD, P], BF16, tag="xt", name=f"xt{g}")
            nc.sync.dma_start_transpose(xt[:], xbt[g][:])
            xkv = xt[:].rearrange("p (b k) c -> p k b c", k=KD)
            xslices[e][s] = [xkv[:, k] for k in range(KD)]

        LOOK = 8  # in-loop staging lookahead (slabs)

        # ---- prologue: expert 0's weights, then full staging chains for
        # the first LOOK slabs (loads on the sync queue - idle at startup,
        # while the Act queue carries w0).
        w = [None] * E_PER
        w[0] = load_weights(0)
        for g in range(LOOK):
            stage_load(g, nc.sync)
            stage_cast(g)
            stage_tpose(g)

        # pending MM2 work: (e, nci, ht_tiles) of the previous chunk round
        pend = None

        def run_mm2(p_e, p_nci, p_ht, last):
            wproj_t = w[p_e][1]
            psos = [
                ps_o.tile([P, 2 * D], F32, tag="pso",
                          name=f"pso{p_e}_{p_nci}_{i}")
                for i in range(NS)
            ]
            ob = out_p.tile([P, NS, D], F32, tag="ob")
            order = (
                [(s, k) for s in range(NS) for k in range(KH)]
                if last else
                [(s, k) for k in range(KH) for s in range(NS)]
            )
            for s, k in order:
                nc.tensor.matmul(
                    psos[s][:, :D],
                    p_ht[k // HPACK][:, k % HPACK, s * P:(s + 1) * P],
                    wproj_t[:, k, :],
                    start=(k == 0),
                    stop=(k == KH - 1),
                )
                if last and k == KH - 1:
                    # final round: per-subtile eviction+store so the output
                    # tail overlaps the last matmuls. The whole output path
                    # (PSUM eviction + store) lives on the otherwise-idle
                    # gpsimd engine/SWDGE queue, so it never queues behind
                    # the x-staging casts (DVE) or loads (Act/sync queues).
                    nc.vector.tensor_copy(ob[:, s, :], psos[s][:, :D])
                    nc.gpsimd.dma_start(
                        out[p_e, p_nci * NC_CHUNK + s * P:
                            p_nci * NC_CHUNK + (s + 1) * P, :],
                        ob[:, s, :],
                    )
            if not last:
                for s, pso in enumerate(psos):
                    nc.vector.tensor_copy(ob[:, s, :], pso[:, :D])
                csl = slice(p_nci * NC_CHUNK, (p_nci + 1) * NC_CHUNK)
                nc.gpsimd.dma_start(
                    out[p_e, csl, :].rearrange("(s p) d -> p s d", p=P), ob[:]
                )

        for t in range(T_CHUNKS + 1):
            if t < T_CHUNKS:
                e, nci = divmod(t, N_CHUNKS)
                if nci == 2 and e + 1 < E_PER:
                    w[e + 1] = load_weights(e + 1)
                wfc_t = w[e][0]
                # staggered staging for upcoming slabs
                if t + LOOK < T_CHUNKS:
                    stage_load(t + LOOK, nc.scalar)
                if LOOK <= t + LOOK - 1 < T_CHUNKS:
                    stage_cast(t + LOOK - 1)
                if LOOK <= t + LOOK - 2 < T_CHUNKS:
                    stage_tpose(t + LOOK - 2)
                # ---- MM1 -> GELU for chunk t ----
                # MM1 accumulates HPACK h_tiles into one 2-bank PSUM tile so
                # GELU evicts in wide ACTIVATE calls; hT is written bf16 so
                # MM2's per-matmul weight loads run at 2-byte FWL speed.
                xk = xslices[e][nci]
                ht_tiles = []
                for hp in range(H_TILES // HPACK):
                    psh = ps_h.tile([P, HPACK, NC_CHUNK], F32, tag="psh")
                    for j in range(HPACK):
                        hi = hp * HPACK + j
                        for k in range(KD):
                            nc.tensor.matmul(
                                psh[:, j, :],
                                wfc_t[:, k, hi * P:(hi + 1) * P],
                                xk[k],
                                start=(k == 0),
                                stop=(k == KD - 1),
                            )
                    ht = ht_p.tile([P, HPACK, NC_CHUNK], BF16, tag="ht")
                    nc.scalar.activation(
                        ht[:], psh[:], mybir.ActivationFunctionType.Gelu
                    )
                    ht_tiles.append(ht)

            # ---- MM2 for the previous chunk round (one-chunk delay: its
            # GELUs completed during this round's MM1, so the PE never
            # waits on the ACT engine) ----
            if pend is not None:
                p_e, p_nci, p_ht = pend
                run_mm2(p_e, p_nci, p_ht, last=(t == T_CHUNKS))
            pend = (e, nci, ht_tiles) if t < T_CHUNKS else None

    _fix_waits(nc)
    return nc


_CACHE = {}


def _get_nc():
    if "nc" not in _CACHE:
        _CACHE["nc"] = _build()
    return _CACHE["nc"]


def kernel(x, w_fc, w_proj, trace=False):
    assert x.shape == (E, CAP, D) and w_fc.shape == (E, D, H)
    assert w_proj.shape == (E, H, D)
    nc = _get_nc()
    x = np.ascontiguousarray(x, dtype=np.float32)
    w_fc = np.ascontiguousarray(w_fc, dtype=np.float32)
    w_proj = np.ascontiguousarray(w_proj, dtype=np.float32)
    in_maps = [
        {
            "x": x[i * E_PER:(i + 1) * E_PER],
            "w_fc": w_fc[i * E_PER:(i + 1) * E_PER],
            "w_proj": w_proj[i * E_PER:(i + 1) * E_PER],
        }
        for i in range(N_CORES)
    ]
    res = run_bass_kernel_spmd(nc, in_maps, list(range(N_CORES)), trace=trace)
    out = np.concatenate([r["out"] for r in res.results], axis=0)
    if trace:
        kernel.last_results = res
    return out


# revision 39
# speedup vs baseline: 1.2353x; 1.0039x over previous
"""Expert-parallel MoE MLP (ExpertMLP) Bass kernel for 8 Trainium2 NeuronCores.

Problem: x[32,4096,256] @ w_fc[32,256,1024] -> gelu(erf) -> @ w_proj[32,1024,256].

Sharding: expert-parallel. Each of the 8 cores gets 4 experts (slices of the
leading axis of every tensor); no cross-core communication. Inside a core, per
expert e:

  1. x[e] ([4096,256], capacity-major) is cast to bf16 (DRAM->DRAM SWDGE cast
     on gpsimd) in 512-row slabs, then each slab is XBar DMA-transposed into
     SBUF as xT [d, c] so the d-contraction of MM1 lies on the partition axis.
     The PE never spends a cycle on transposes.
  2. MM1: hT[h_tile, c_chunk] += w_fc_tile.T @ xT_chunk - w_fc's natural
     [d, h] layout is the stationary operand, so it needs no transpose.
  3. GELU (exact erf form) runs on the ACT engine as the PSUM->SBUF eviction,
     writing bf16 hT tiles.
  4. MM2 uses hT slices as the *stationary* operand and w_proj's natural
     [h, d] layout as the moving operand: out[c_sub, d] += hT_slice.T @
     w_proj_tile. The result lands directly in [capacity, d] orientation, so
     no output transpose is needed.

Scheduling (what makes it fast):
  - Priority-ordered prologue: w_fc[e0] (scalar HWDGE queue) and the cast+
    transpose chain for e0's first slab are enqueued before everything else,
    so MM1 starts ~13us in instead of waiting ~48us for all casts to drain.
  - MM2 of chunk t runs after MM1 of chunk t+1 (one-chunk software pipeline),
    so the ACT-engine GELU of chunk t completes long before MM2 needs it and
    the PE never stalls on the activation.
  - Queue separation: weights go on the Act HWDGE queue, x-casts on the
    gpsimd SWDGE queue, transposes + output stores on the sync HWDGE queue.
    Transposes/casts for expert e+2 are issued inside expert e's loop so
    tile-pool aliasing never head-of-line-blocks the store stream.
"""

import numpy as np
from contextlib import ExitStack

import bass_rust as _br
import concourse.bass as bass
import concourse.tile as tile
from concourse import mybir
from concourse.bass_utils import run_bass_kernel_spmd

E, CAP, D, H = 32, 4096, 256, 1024
N_CORES = 8
E_PER = E // N_CORES  # 4 experts per core
P = 128
F32 = mybir.dt.float32
F32R = mybir.dt.float32r
BF16 = mybir.dt.bfloat16

KD = D // P        # 2 k-tiles in MM1's contraction
KH = H // P        # 8 k-tiles in MM2's contraction
NC_CHUNK = 512     # capacity chunk processed per MM1/MM2 round == slab size
N_CHUNKS = CAP // NC_CHUNK
H_TILES = H // P
HPACK = 2          # h_tiles packed per PSUM tile / GELU call
NS = NC_CHUNK // P
T_CHUNKS = E_PER * N_CHUNKS  # 32 global (expert, chunk) rounds


def _fix_waits(nc):
    """walrus here accepts only one sync wait per instruction; hoist excess
    waits onto standalone EventSemaphore instructions inserted before the
    offender (same engine => same sequencer order)."""
    for fn in nc.m.functions:
        for bb in fn.blocks:
            new = []
            changed = False
            for inst in bb.instructions:
                si = inst.sync_info
                if si is not None and len(si.on_wait) > 1:
                    waits = list(si.on_wait)
                    for w in waits[:-1]:
                        ev = mybir.InstEventSemaphore(
                            name=nc.get_next_instruction_name()
                        )
                        ev.engine = inst.engine
                        ev.sync_info = _br.SyncInfo(on_wait=[w], on_update=[])
                        nc.register_instruction(ev)
                        new.append(ev)
                    inst.sync_info = _br.SyncInfo(
                        on_wait=waits[-1:], on_update=list(si.on_update)
                    )
                    changed = True
                new.append(inst)
            if changed:
                bb.instructions = new


def _build():
    nc = bass.Bass(trn_type="TRN2", target_bir_lowering=False, debug=False)
    x = nc.dram_tensor("x", [E_PER, CAP, D], F32, kind="ExternalInput").ap()
    w_fc = nc.dram_tensor("w_fc", [E_PER, D, H], F32, kind="ExternalInput").ap()
    w_proj = nc.dram_tensor("w_proj", [E_PER, H, D], F32, kind="ExternalInput").ap()
    out = nc.dram_tensor("out", [E_PER, CAP, D], F32, kind="ExternalOutput").ap()

    with tile.TileContext(nc) as tc, ExitStack() as ctx:
        # 2 experts' worth of xT slabs in flight; ring aliasing gates the
        # transposes of expert e+2 on MM1 of expert e having consumed the slab.
        # x staging pools, one 512-row slab at a time:
        #   xsf: f32 slab straight off HBM     [128, NS, 256] (4 KB/part)
        #   xsb: bf16 copy (DVE cast)          [128, NS, 256] (2 KB/part)
        #   xtp: XBar-transposed xT blocks     [128, NS*KD, 128] (2 KB/part)
        # Ring aliasing paces the loads: load i+6 waits for cast i, cast i+6
        # waits for transpose i, transpose i+16 waits for MM1 reads of the
        # expert two ahead.
        xsf = ctx.enter_context(tc.tile_pool(name="xsf", bufs=5))
        xsb = ctx.enter_context(tc.tile_pool(name="xsb", bufs=5))
        xtp = ctx.enter_context(tc.tile_pool(name="xtp", bufs=8))
        wload = ctx.enter_context(tc.tile_pool(name="wload", bufs=2))
        wfc_p = ctx.enter_context(tc.tile_pool(name="wfc", bufs=2))
        wproj_p = ctx.enter_context(tc.tile_pool(name="wproj", bufs=2))
        ht_p = ctx.enter_context(tc.tile_pool(name="ht", bufs=8))
        out_p = ctx.enter_context(tc.tile_pool(name="outp", bufs=3))
        ps_h = ctx.enter_context(tc.tile_pool(name="ps_h", bufs=2, space="PSUM"))
        ps_o = ctx.enter_context(tc.tile_pool(name="ps_o", bufs=4, space="PSUM"))

        def load_weights(e):
            # raw f32 loads on the Act HWDGE queue (kept clear of the cast
            # and transpose streams), cast to bf16 on the idle DVE.
            wfc_raw = wload.tile([P, KD, H], F32, tag="wl")
            nc.scalar.dma_start(wfc_raw[:], w_fc[e].rearrange("(k p) h -> p k h", p=P))
            wfc = wfc_p.tile([P, KD, H], BF16, tag="wfc")
            nc.vector.tensor_copy(wfc[:], wfc_raw[:])
            wproj_raw = wload.tile([P, KH, D], F32, tag="wl")
            nc.scalar.dma_start(
                wproj_raw[:], w_proj[e].rearrange("(k p) d -> p k d", p=P)
            )
            wproj = wproj_p.tile([P, KH, D], BF16, tag="wproj")
            nc.vector.tensor_copy(wproj[:], wproj_raw[:])
            return wfc, wproj

        # x staging, one 512-row slab at a time, in three engine-disjoint
        # stages (global slab index g = e*N_CHUNKS + s = chunk index):
        #   load: DMA the f32 slab to SBUF partition-blocked [128, b, 256]
        #   cast: f32 -> bf16 on the DVE
        #   tpose: ONE wide SBUF->SBUF XBar transpose [128, 1024] ->
        #          [128, (b k), 128] - each 128-column group of the source
        #          becomes one transposed output block, so all NS*KD
        #          128x128 blocks land in one instruction, and the transpose
        #          never touches HBM.
        # MM1 then reads k-tile views [128, b, 128] (strided middle dim).
        # Staging is issued IN-LOOP, staggered (load g+8 / cast g+7 /
        # transpose g+6 at chunk g): every in-order engine sees staging
        # interleaved with its consumption-time work, so no long prologue
        # stream ever blocks a later instruction on the same engine.
        xslices = [[None] * N_CHUNKS for _ in range(E_PER)]
        xbt = [None] * (T_CHUNKS // 2)
        SNS = 2 * NS  # row-blocks per (1024-row) staging slab

        def stage_load(g, queue):
            # slab g covers chunks 2g, 2g+1 (1024 capacity rows)
            e, s2 = divmod(g, N_CHUNKS // 2)
            rs = slice(s2 * 2 * NC_CHUNK, (s2 + 1) * 2 * NC_CHUNK)
            xf = xsf.tile([P, SNS, D], F32, tag="xf", name=f"xf{g}")
            queue.dma_start(xf[:], x[e][rs].rearrange("(b p) d -> p b d", p=P))
            xbt[g] = xf

        def stage_cast(g):
            xb = xsb.tile([P, SNS, D], BF16, tag="xb", name=f"xb{g}")
            nc.vector.tensor_copy(xb[:], xbt[g][:])
            xbt[g] = xb

        def stage_tpose(g):
            e, s2 = divmod(g, N_CHUNKS // 2)
            xt = xtp.tile([P, SNS * KD, P], BF16, tag="xt", name=f"xt{g}")
            nc.sync.dma_start_transpose(xt[:], xbt[g][:])
            xkv = xt[:].rearrange("p (b k) c -> p k b c", k=KD)
            for half in range(2):
                xslices[e][s2 * 2 + half] = [
                    xkv[:, k, half * NS:(half + 1) * NS] for k in range(KD)
                ]

        LOOK = 4  # in-loop staging lookahead (1024-row slabs = 2 chunks each)

        # ---- prologue: expert 0's weights, then full staging chains for
        # the first LOOK slabs (loads on the sync queue - idle at startup,
        # while the Act queue carries w0).
        w = [None] * E_PER
        w[0] = load_weights(0)
        for g in range(LOOK):
            stage_load(g, nc.sync)
            stage_cast(g)
            stage_tpose(g)

        # pending MM2 work: (e, nci, ht_tiles) of the previous chunk round
        pend = None

        def run_mm2(p_e, p_nci, p_ht, last):
            wproj_t = w[p_e][1]
            psos = [
                ps_o.tile([P, 2 * D], F32, tag="pso",
                          name=f"pso{p_e}_{p_nci}_{i}")
                for i in range(NS)
            ]
            ob = out_p.tile([P, NS, D], F32, tag="ob")
            order = (
                [(s, k) for s in range(NS) for k in range(KH)]
                if last else
                [(s, k) for k in range(KH) for s in range(NS)]
            )
            for s, k in order:
                nc.tensor.matmul(
                    psos[s][:, :D],
                    p_ht[k // HPACK][:, k % HPACK, s * P:(s + 1) * P],
                    wproj_t[:, k, :],
                    start=(k == 0),
                    stop=(k == KH - 1),
                )
                if last and k == KH - 1:
                    # final round: per-subtile eviction+store so the output
                    # tail overlaps the last matmuls. The whole output path
                    # (PSUM eviction + store) lives on the otherwise-idle
                    # gpsimd engine/SWDGE queue, so it never queues behind
                    # the x-staging casts (DVE) or loads (Act/sync queues).
                    nc.vector.tensor_copy(ob[:, s, :], psos[s][:, :D])
                    nc.gpsimd.dma_start(
                        out[p_e, p_nci * NC_CHUNK + s * P:
                            p_nci * NC_CHUNK + (s + 1) * P, :],
                        ob[:, s, :],
                    )
            if not last:
                for s, pso in enumerate(psos):
                    nc.vector.tensor_copy(ob[:, s, :], pso[:, :D])
                csl = slice(p_nci * NC_CHUNK, (p_nci + 1) * NC_CHUNK)
                nc.gpsimd.dma_start(
                    out[p_e, csl, :].rearrange("(s p) d -> p s d", p=P), ob[:]
                )

        for t in range(T_CHUNKS + 1):
            if t < T_CHUNKS:
                e, nci = divmod(t, N_CHUNKS)
                if nci == 2 and e + 1 < E_PER:
                    w[e + 1] = load_weights(e + 1)
                wfc_t = w[e][0]
                # staggered staging for upcoming 1024-row slabs: slab g
                # covers chunks 2g..2g+1; stage one pipeline step every
                # other chunk
                if t % 2 == 0:
                    g = t // 2
                    if g + LOOK < T_CHUNKS // 2:
                        stage_load(g + LOOK, nc.scalar)
                    if LOOK <= g + LOOK - 1 < T_CHUNKS // 2:
                        stage_cast(g + LOOK - 1)
                    if LOOK <= g + LOOK - 2 < T_CHUNKS // 2:
                        stage_tpose(g + LOOK - 2)
                # ---- MM1 -> GELU for chunk t ----
                # MM1 accumulates HPACK h_tiles into one 2-bank PSUM tile so
                # GELU evicts in wide ACTIVATE calls; hT is written bf16 so
                # MM2's per-matmul weight loads run at 2-byte FWL speed.
                xk = xslices[e][nci]
                ht_tiles = []
                for hp in range(H_TILES // HPACK):
                    psh = ps_h.tile([P, HPACK, NC_CHUNK], F32, tag="psh")
                    for j in range(HPACK):
                        hi = hp * HPACK + j
                        for k in range(KD):
                            nc.tensor.matmul(
                                psh[:, j, :],
                                wfc_t[:, k, hi * P:(hi + 1) * P],
                                xk[k],
                                start=(k == 0),
                                stop=(k == KD - 1),
                            )
                    ht = ht_p.tile([P, HPACK, NC_CHUNK], BF16, tag="ht")
                    nc.scalar.activation(
                        ht[:], psh[:], mybir.ActivationFunctionType.Gelu
                    )
                    ht_tiles.append(ht)

            # ---- MM2 for the previous chunk round (one-chunk delay: its
            # GELUs completed during this round's MM1, so the PE never
            # waits on the ACT engine) ----
            if pend is not None:
                p_e, p_nci, p_ht = pend
                run_mm2(p_e, p_nci, p_ht, last=(t == T_CHUNKS))
            pend = (e, nci, ht_tiles) if t < T_CHUNKS else None

    _fix_waits(nc)
    return nc


_CACHE = {}


def _get_nc():
    if "nc" not in _CACHE:
        _CACHE["nc"] = _build()
    return _CACHE["nc"]


def kernel(x, w_fc, w_proj, trace=False):
    assert x.shape == (E, CAP, D) and w_fc.shape == (E, D, H)
    assert w_proj.shape == (E, H, D)
    nc = _get_nc()
    x = np.ascontiguousarray(x, dtype=np.float32)
    w_fc = np.ascontiguousarray(w_fc, dtype=np.float32)
    w_proj = np.ascontiguousarray(w_proj, dtype=np.float32)
    in_maps = [
        {
            "x": x[i * E_PER:(i + 1) * E_PER],
            "w_fc": w_fc[i * E_PER:(i + 1) * E_PER],
            "w_proj": w_proj[i * E_PER:(i + 1) * E_PER],
        }
        for i in range(N_CORES)
    ]
    res = run_bass_kernel_spmd(nc, in_maps, list(range(N_CORES)), trace=trace)
    out = np.concatenate([r["out"] for r in res.results], axis=0)
    if trace:
        kernel.last_results = res
    return out
